# revision 8
# baseline (speedup 1.0000x reference)
"""Trainium2 Bass kernel for nn_CERLoss (CER / Levenshtein DP loss).

Strategy (8 NeuronCores, data-parallel over batch; v2 pipelined design):
  - Each core owns 4 batch rows ([4, 256, 32000] fp32 slab).
  - Streaming is s-interleaved: block sigma covers s in [32*sigma, 32*sigma+32)
    for ALL 4 b rows (partition p = b*32 + s_local), so the argmax for pred
    positions arrives in j-order for every batch row simultaneously.
  - Per block: 10 chunk DMAs [128, 3200] fp32; per chunk one 3D sub-reduce
    [128, 25, 128] -> 25 sub-maxes; rowmax + custom FIRSTIDX finds the first
    128-wide sub attaining the row max (exact fp32); indirect-DMA refetch of
    just that sub ([128, 128] fp32, 0.4% extra traffic); max8 + max_index give
    the exact first-occurrence argmax.
  - The DP is scanned over PRED positions j (transposed vs the reference's
    scan over target i; the recurrences are symmetric) so DP steps can run as
    soon as argmax_j is known: full overlap with streaming. Mismatch rows
    G[b, j, i] = (t_i != idx_j) + 512 - 514*w_i are built 32 at a time with a
    custom fused ne-add DVE op and staged into per-b layout via SBUF->SBUF DMA.
  - Each DP step is ONE custom DVE instruction (DPSTEP): a hand-edited uop
    program that computes out[t] = min(state, cur[t], cur[t-1] + G_j[t-1])
    at 1 element/cycle using a CURR_ALU_OUT delay-lane capture, with the
    running min (scan) and the step's final value (accum_out -> R[:, j])
    emitted in the same instruction. S[j][i] = D[j][i] - j - C_i stays
    integral with |.| <= 1536, exact in fp16.
  - loss_row = R[len_b] + 2*len_b; host averages the 32 per-row losses.
  - Software pipelining: DP steps for block sigma-1 are emitted between the
    select and the resolve of block sigma, hiding the refetch DMA latency.
"""

import copy
import numpy as np

B, S, V = 32, 256, 32000
NCORES = 8
BC = B // NCORES            # batch rows per core = 4
SW = 32                     # s-positions per stream block
NBLK = S // SW              # 8 stream blocks
VC = 3200                   # vocab chunk (floats)
NCH = V // VC               # 10 chunks per block
SUB = 128                   # sub-chunk for argmax refetch
NSUB = V // SUB             # 250 subs per (b,s) row
SPC = VC // SUB             # 25 subs per chunk
BIG = 512.0
GW = S + 2                  # 258-wide DP rows (sentinel col 0, pad col 257)
J1 = S + 1                  # 257 memo columns

_cache = {}
DEBUG = False


def _register_custom_ops():
    import concourse.dve_ops as dve_ops
    from concourse.dve_ops import DveOp, DveOpSpec, has_src1
    from concourse.dve_spec import (Spec, Src0, Src1, C0, C1, Idx, scan, minn,
                                    select, eq, ne, AluOp, lower)
    from concourse.dve_uop import AluInp, DelayInp

    def reg(name, spec, edit=None):
        for o in dve_ops.OPS:
            if o.name == name:
                return o
        row = max(dve_ops._SUB_OPCODE_FOR_NAME.values()) + 1
        dve_ops._SUB_OPCODE_FOR_NAME[name] = row
        shas = {}
        for ver in ("v3", "v4"):
            uops = [copy.deepcopy(u) for u in lower(spec, ver=ver)]
            if edit is not None:
                edit(uops)
            tmp = DveOpSpec(name=name, opcode=row, uops=uops,
                            rd1_en=has_src1(spec))
            shas[ver] = tmp.sha(ver)
            dve_ops._COMPILE_CACHE[(name, ver)] = tmp
        op = DveOp(name, spec, subdim=False, uops_sha=shas)
        dve_ops.OPS.append(op)
        dve_ops.CUSTOM_DVE_SPECS[name] = spec
        return op

    # out[t] = state_t = min(state_{t-1}, in0[t], a_{t-1}); a_t = in1[t]+s1;
    # a_{-1} = state_{-1} = s0; accum_out = final state.
    def dpstep_edit(uops):
        seed, steady = uops
        st0 = seed.datapath_config[0]
        st0.op = AluOp.BYPASS
        st0.alu_src0 = AluInp.PREV_DELAY_3
        st0.alu_src1 = AluInp.PREV_DELAY_3
        st0s = steady.datapath_config[0]
        st0s.alu_src0 = AluInp.PREV_DELAY_0
        st0s.alu_src1 = AluInp.PREV_DELAY_1
        st0s.delay[4] = DelayInp.CURR_ALU_OUT
        st0s.delay_enable[4] = 1
        st1s = steady.datapath_config[1]
        st1s.alu_src1 = AluInp.PREV_DELAY_4

    DPSTEP = reg("DPSTEP_ANT", Spec(
        body=scan(AluOp.MIN, minn(Src0, Src1 + C1), init=C0),
        accum=AluOp.MIN, accum_init=C0,
        reference=lambda in0, in1, s0, s1, imm2: in0,
    ), edit=dpstep_edit)

    # first index k where in0[k] == s0 (else s1); accum MIN -> first idx
    FIRSTIDX = reg("FIRSTIDX_ANT", Spec(
        body=select(eq(Src0, C0), Idx, C1),
        accum=AluOp.MIN, accum_init=C1,
        reference=lambda in0, in1, s0, s1, imm2: np.where(
            in0 == s0, np.arange(in0.shape[-1], dtype=np.float32), s1),
    ))

    # out = (in0 != s0) + in1   (mismatch + base, fused)
    NEADD = reg("NEADD_ANT", Spec(
        body=ne(Src0, C0) + Src1,
        reference=lambda in0, in1, s0, s1, imm2: (in0 != s0) + in1,
    ))
    return DPSTEP, FIRSTIDX, NEADD


def _build():
    import sys
    if '/opt/trn_rl_repo' not in sys.path:
        sys.path.insert(0, '/opt/trn_rl_repo')
    import concourse.bass as bass
    import concourse.bacc as bacc
    import concourse.mybir as mybir
    import concourse.tile as tile

    DPSTEP, FIRSTIDX, NEADD = _register_custom_ops()

    fp32 = mybir.dt.float32
    fp16 = mybir.dt.float16
    i32 = mybir.dt.int32
    u32 = mybir.dt.uint32
    Alu = mybir.AluOpType
    AX = mybir.AxisListType.X

    nc = bacc.Bacc(None, target_bir_lowering=False, debug=False)
    x = nc.dram_tensor("input", [BC, S, V], fp32, kind="ExternalInput")
    tg = nc.dram_tensor("target", [BC, S], fp32, kind="ExternalInput")
    out = nc.dram_tensor("loss_part", [BC, 1], fp32, kind="ExternalOutput")
    if DEBUG:
        dbg_t = nc.dram_tensor("dbg_t", [128, S], fp32, kind="ExternalOutput")
        dbg_b = nc.dram_tensor("dbg_b", [128, S], fp32, kind="ExternalOutput")
        dbg_i = nc.dram_tensor("dbg_i", [128, NBLK], fp32, kind="ExternalOutput")
        dbg_g = nc.dram_tensor("dbg_g", [BC, 4 * GW], fp32, kind="ExternalOutput")
        dbg_R = nc.dram_tensor("dbg_R", [BC, J1], fp32, kind="ExternalOutput")
    scratch = nc.dram_tensor("rowbase_scratch", [BC, SW], fp32, kind="Internal")

    # refetch view: one row per 128-float sub
    x_subs = x[:, :, :].rearrange("b s (c v) -> (b s c) v", v=SUB)  # [256000,128]

    with tile.TileContext(nc) as tc:
        with tc.tile_pool(name="persist", bufs=1) as cp, \
             tc.tile_pool(name="chunks", bufs=3) as chp, \
             tc.tile_pool(name="work", bufs=2) as wp:

            # ---------------- one-time setup ----------------
            # rowbase[p] = b*64000 + s_local*250  (p = b*32 + s_local)
            T = cp.tile([SW, BC], i32, tag="T")
            nc.gpsimd.iota(T[:, :], pattern=[[S, BC]], base=0,
                           channel_multiplier=1)
            Tf = cp.tile([SW, BC], fp32, tag="Tf")
            nc.vector.tensor_scalar(out=Tf[:, :], in0=T[:, :],
                                    scalar1=float(NSUB), scalar2=None,
                                    op0=Alu.mult)
            nc.sync.dma_start(out=scratch[:, :].rearrange("b s -> s b"),
                              in_=Tf[:, :])
            rowb_f = cp.tile([128, 1], fp32, tag="rowb_f")
            nc.sync.dma_start(
                out=rowb_f[:, :],
                in_=scratch[:, :].rearrange("b (s u) -> (b s) u", u=1))

            # target broadcast [128, 256]: row p=(b, j_local) -> target[b, :]
            t_bcast = cp.tile([128, S], fp32, tag="t_bcast")
            for b in range(BC):
                nc.sync.dma_start(
                    out=t_bcast[SW * b:SW * (b + 1), :],
                    in_=tg[b:b + 1, :].to_broadcast([SW, S]))
            # base[p, i] = 512 - 514*w_i(b)
            wrow = cp.tile([128, S], fp32, tag="wrow")
            nc.vector.tensor_scalar(out=wrow[:, :], in0=t_bcast[:, :],
                                    scalar1=0.0, scalar2=None,
                                    op0=Alu.not_equal)
            base_bc = cp.tile([128, S], fp16, tag="base_bc")
            nc.vector.tensor_scalar(out=base_bc[:, :], in0=wrow[:, :],
                                    scalar1=-514.0, scalar2=BIG,
                                    op0=Alu.mult, op1=Alu.add)

            # G rows [4, 256 * 258] fp16; col0 = col257 = BIG
            G = cp.tile([BC, S * GW], fp16, tag="G")
            G3 = G[:, :].rearrange("p (j i) -> p j i", i=GW)
            nc.vector.memset(G3[:, :, 0:1], BIG)
            nc.vector.memset(G3[:, :, GW - 1:GW], BIG)

            # DP state
            sa = cp.tile([BC, GW], fp16, tag="sa")
            sb = cp.tile([BC, GW], fp16, tag="sb")
            nc.vector.memset(sa[:, :], 0.0)
            nc.vector.memset(sa[:, 0:1], BIG)
            R = cp.tile([BC, J1], fp16, tag="R")
            nc.vector.memset(R[:, 0:1], 0.0)
            cur, nxt = sa, sb

            # per-sigma staged state for the software pipeline
            stage = {}

            def emit_stream(k):
                mall = wp.tile([128, NSUB], fp32, name="mall", tag="mall")
                for c in range(NCH):
                    ch = chp.tile([128, VC], fp32, tag="ch")
                    nc.sync.dma_start(
                        out=ch[:, :],
                        in_=x[:, SW * k:SW * (k + 1), VC * c:VC * (c + 1)])
                    ch3 = ch[:, :].rearrange("p (s v) -> p s v", v=SUB)
                    nc.vector.tensor_reduce(out=mall[:, SPC * c:SPC * (c + 1)],
                                            in_=ch3, axis=AX, op=Alu.max)
                return mall

            def emit_select(k, mall):
                rmax = wp.tile([128, 1], fp32, tag="rmax")
                nc.vector.tensor_reduce(out=rmax[:, :], in_=mall[:, :],
                                        axis=AX, op=Alu.max)
                fsc = wp.tile([128, NSUB], fp32, tag="fsc")
                f = wp.tile([128, 1], fp32, tag="f")
                nc.vector._custom_dve(FIRSTIDX, out=fsc[:, :], in0=mall[:, :],
                                      s0=rmax[:, :1], s1=1000.0,
                                      accum_out=f[:, :])
                fetchf = wp.tile([128, 1], fp32, tag="fetchf")
                nc.vector.tensor_scalar(out=fetchf[:, :], in0=f[:, :],
                                        scalar1=float(k * SW * NSUB),
                                        scalar2=rowb_f[:, :1],
                                        op0=Alu.add, op1=Alu.add)
                fetch = wp.tile([128, 1], i32, tag="fetch")
                nc.vector.tensor_copy(out=fetch[:, :], in_=fetchf[:, :])
                refetch = wp.tile([128, SUB], fp32, tag="refetch")
                nc.gpsimd.indirect_dma_start(
                    out=refetch[:, :], out_offset=None,
                    in_=x_subs[:, :],
                    in_offset=bass.IndirectOffsetOnAxis(ap=fetch[:, :1], axis=0))
                return f, refetch

            def emit_resolve(k, f, refetch):
                m8 = wp.tile([128, 8], fp32, tag="m8")
                nc.vector.max(out=m8[:, :], in_=refetch[:, :])
                i8 = wp.tile([128, 8], u32, tag="i8")
                nc.vector.max_index(out=i8[:, :], in_max=m8[:, :],
                                    in_values=refetch[:, :])
                idxf = wp.tile([128, 1], fp32, tag="idxf")
                nc.vector.tensor_copy(out=idxf[:, :], in_=i8[:, 0:1])
                idxg = wp.tile([128, 1], fp32, tag="idxg")
                nc.vector.tensor_scalar(out=idxg[:, :], in0=f[:, :],
                                        scalar1=float(SUB),
                                        scalar2=idxf[:, :1],
                                        op0=Alu.mult, op1=Alu.add)
                mt = wp.tile([128, S], fp16, tag="mt")
                nc.vector._custom_dve(NEADD, out=mt[:, :], in0=t_bcast[:, :],
                                      in1=base_bc[:, :], s0=idxg[:, :1],
                                      s1=0.0)
                if DEBUG:
                    nc.sync.dma_start(out=dbg_i[:, k:k + 1], in_=idxg[:, :])
                nc.sync.dma_start(
                    out=G3[:, SW * k:SW * (k + 1), 1:S + 1],
                    in_=mt[:, :])

            def emit_dp(k):
                nonlocal cur, nxt
                for j in range(SW * k + 1, SW * (k + 1) + 1):
                    g = G[:, (j - 1) * GW:(j - 1) * GW + GW]
                    nc.vector._custom_dve(DPSTEP, out=nxt[:, :],
                                          in0=cur[:, :], in1=g,
                                          s0=BIG, s1=0.0,
                                          accum_out=R[:, j:j + 1])
                    cur, nxt = nxt, cur

            # ---------------- pipelined main loop ----------------
            for k in range(NBLK):
                mall = emit_stream(k)
                f, refetch = emit_select(k, mall)
                if k > 0:
                    emit_dp(k - 1)
                emit_resolve(k, f, refetch)
            emit_dp(NBLK - 1)

            if DEBUG:
                dtt = cp.tile([128, S], fp32, tag="dtt")
                nc.vector.tensor_copy(out=dtt[:, :], in_=t_bcast[:, :])
                nc.sync.dma_start(out=dbg_t[:, :], in_=dtt[:, :])
                dbb = cp.tile([128, S], fp32, tag="dbb")
                nc.vector.tensor_copy(out=dbb[:, :], in_=base_bc[:, :])
                nc.sync.dma_start(out=dbg_b[:, :], in_=dbb[:, :])
                dgg = cp.tile([BC, 4 * GW], fp32, tag="dgg")
                for di, j in enumerate((0, 1, 100, 255)):
                    nc.vector.tensor_copy(
                        out=dgg[:, di * GW:(di + 1) * GW],
                        in_=G[:, j * GW:(j + 1) * GW])
                nc.sync.dma_start(out=dbg_g[:, :], in_=dgg[:, :])
                dRR = cp.tile([BC, J1], fp32, tag="dRR")
                nc.vector.tensor_copy(out=dRR[:, :], in_=R[:, :])
                nc.sync.dma_start(out=dbg_R[:, :], in_=dRR[:, :])

            # ---------------- extraction ----------------
            tg4 = cp.tile([BC, S], fp32, tag="tg4")
            nc.sync.dma_start(out=tg4[:, :], in_=tg[:, :])
            w4 = cp.tile([BC, S], fp32, tag="w4")
            nc.vector.tensor_scalar(out=w4[:, :], in0=tg4[:, :],
                                    scalar1=0.0, scalar2=None,
                                    op0=Alu.not_equal)
            lenr = cp.tile([BC, 1], fp32, tag="lenr")
            nc.vector.tensor_reduce(out=lenr[:, :], in_=w4[:, :],
                                    axis=AX, op=Alu.add)
            len2 = cp.tile([BC, 1], fp32, tag="len2")
            nc.vector.tensor_scalar(out=len2[:, :], in0=lenr[:, :],
                                    scalar1=2.0, scalar2=None, op0=Alu.mult)
            iotai = cp.tile([BC, J1], i32, tag="iotai")
            nc.gpsimd.iota(iotai[:, :], pattern=[[1, J1]], base=0,
                           channel_multiplier=0)
            iota_f = cp.tile([BC, J1], fp32, tag="iota_f")
            nc.vector.tensor_copy(out=iota_f[:, :], in_=iotai[:, :])
            eqj = cp.tile([BC, J1], fp32, tag="eqj")
            nc.vector.tensor_scalar(out=eqj[:, :], in0=iota_f[:, :],
                                    scalar1=lenr[:, :1], scalar2=None,
                                    op0=Alu.is_equal)
            Rf = cp.tile([BC, J1], fp32, tag="Rf")
            nc.vector.tensor_copy(out=Rf[:, :], in_=R[:, :])
            prod = cp.tile([BC, J1], fp32, tag="prod")
            nc.vector.tensor_tensor(out=prod[:, :], in0=eqj[:, :],
                                    in1=Rf[:, :], op=Alu.mult)
            red = cp.tile([BC, 1], fp32, tag="red")
            nc.vector.tensor_reduce(out=red[:, :], in_=prod[:, :],
                                    axis=AX, op=Alu.add)
            loss = cp.tile([BC, 1], fp32, tag="loss")
            nc.vector.tensor_scalar(out=loss[:, :], in0=red[:, :],
                                    scalar1=len2[:, :1], scalar2=None,
                                    op0=Alu.add)
            nc.sync.dma_start(out=out[:, :], in_=loss[:, :])

    nc.compile()
    return nc


def kernel(input, target):
    import sys
    if '/opt/trn_rl_repo' not in sys.path:
        sys.path.insert(0, '/opt/trn_rl_repo')
    from concourse.bass_utils import run_bass_kernel_spmd

    if 'nc' not in _cache:
        _cache['nc'] = _build()
    nc = _cache['nc']

    input = np.ascontiguousarray(np.asarray(input, dtype=np.float32))
    target_f = np.asarray(target).astype(np.float32)

    in_maps = []
    for c in range(NCORES):
        in_maps.append({
            "input": input[BC * c:BC * (c + 1)],
            "target": np.ascontiguousarray(target_f[BC * c:BC * (c + 1)]),
        })
    res = run_bass_kernel_spmd(nc, in_maps, core_ids=list(range(NCORES)))
    parts = [res.results[c]["loss_part"][:, 0] for c in range(NCORES)]
    losses = np.concatenate(parts)
    return np.float32(losses.mean())


# revision 9
# speedup vs baseline: 1.0009x; 1.0009x over previous
"""Trainium2 Bass kernel for nn_CERLoss (CER / Levenshtein DP loss).

Strategy (8 NeuronCores, data-parallel over batch; v2 pipelined design):
  - Each core owns 4 batch rows ([4, 256, 32000] fp32 slab).
  - Streaming is s-interleaved: block sigma covers s in [32*sigma, 32*sigma+32)
    for ALL 4 b rows (partition p = b*32 + s_local), so the argmax for pred
    positions arrives in j-order for every batch row simultaneously.
  - Per block: 10 chunk DMAs [128, 3200] fp32; per chunk one 3D sub-reduce
    [128, 25, 128] -> 25 sub-maxes; rowmax + custom FIRSTIDX finds the first
    128-wide sub attaining the row max (exact fp32); indirect-DMA refetch of
    just that sub ([128, 128] fp32, 0.4% extra traffic); max8 + max_index give
    the exact first-occurrence argmax.
  - The DP is scanned over PRED positions j (transposed vs the reference's
    scan over target i; the recurrences are symmetric) so DP steps can run as
    soon as argmax_j is known: full overlap with streaming. Mismatch rows
    G[b, j, i] = (t_i != idx_j) + 512 - 514*w_i are built 32 at a time with a
    custom fused ne-add DVE op and staged into per-b layout via SBUF->SBUF DMA.
  - Each DP step is ONE custom DVE instruction (DPSTEP): a hand-edited uop
    program that computes out[t] = min(state, cur[t], cur[t-1] + G_j[t-1])
    at 1 element/cycle using a CURR_ALU_OUT delay-lane capture, with the
    running min (scan) and the step's final value (accum_out -> R[:, j])
    emitted in the same instruction. S[j][i] = D[j][i] - j - C_i stays
    integral with |.| <= 1536, exact in fp16.
  - loss_row = R[len_b] + 2*len_b; host averages the 32 per-row losses.
  - Software pipelining: DP steps for block sigma-1 are emitted between the
    select and the resolve of block sigma, hiding the refetch DMA latency.
"""

import copy
import numpy as np

B, S, V = 32, 256, 32000
NCORES = 8
BC = B // NCORES            # batch rows per core = 4
SW = 32                     # s-positions per stream block
NBLK = S // SW              # 8 stream blocks
VC = 3200                   # vocab chunk (floats)
NCH = V // VC               # 10 chunks per block
SUB = 128                   # sub-chunk for argmax refetch
NSUB = V // SUB             # 250 subs per (b,s) row
SPC = VC // SUB             # 25 subs per chunk
BIG = 512.0
GW = S + 2                  # 258-wide DP rows (sentinel col 0, pad col 257)
J1 = S + 1                  # 257 memo columns

_cache = {}
DEBUG = False


def _register_custom_ops():
    import concourse.dve_ops as dve_ops
    from concourse.dve_ops import DveOp, DveOpSpec, has_src1
    from concourse.dve_spec import (Spec, Src0, Src1, C0, C1, Idx, scan, minn,
                                    select, eq, ne, AluOp, lower)
    from concourse.dve_uop import AluInp, DelayInp

    def reg(name, spec, edit=None):
        for o in dve_ops.OPS:
            if o.name == name:
                return o
        row = max(dve_ops._SUB_OPCODE_FOR_NAME.values()) + 1
        dve_ops._SUB_OPCODE_FOR_NAME[name] = row
        shas = {}
        for ver in ("v3", "v4"):
            uops = [copy.deepcopy(u) for u in lower(spec, ver=ver)]
            if edit is not None:
                edit(uops)
            tmp = DveOpSpec(name=name, opcode=row, uops=uops,
                            rd1_en=has_src1(spec))
            shas[ver] = tmp.sha(ver)
            dve_ops._COMPILE_CACHE[(name, ver)] = tmp
        op = DveOp(name, spec, subdim=False, uops_sha=shas)
        dve_ops.OPS.append(op)
        dve_ops.CUSTOM_DVE_SPECS[name] = spec
        return op

    # out[t] = state_t = min(state_{t-1}, in0[t], a_{t-1}); a_t = in1[t]+s1;
    # a_{-1} = state_{-1} = s0; accum_out = final state.
    def dpstep_edit(uops):
        seed, steady = uops
        st0 = seed.datapath_config[0]
        st0.op = AluOp.BYPASS
        st0.alu_src0 = AluInp.PREV_DELAY_3
        st0.alu_src1 = AluInp.PREV_DELAY_3
        st0s = steady.datapath_config[0]
        st0s.alu_src0 = AluInp.PREV_DELAY_0
        st0s.alu_src1 = AluInp.PREV_DELAY_1
        st0s.delay[4] = DelayInp.CURR_ALU_OUT
        st0s.delay_enable[4] = 1
        st1s = steady.datapath_config[1]
        st1s.alu_src1 = AluInp.PREV_DELAY_4

    DPSTEP = reg("DPSTEP_ANT", Spec(
        body=scan(AluOp.MIN, minn(Src0, Src1 + C1), init=C0),
        accum=AluOp.MIN, accum_init=C0,
        reference=lambda in0, in1, s0, s1, imm2: in0,
    ), edit=dpstep_edit)

    # first index k where in0[k] == s0 (else s1); accum MIN -> first idx
    FIRSTIDX = reg("FIRSTIDX_ANT", Spec(
        body=select(eq(Src0, C0), Idx, C1),
        accum=AluOp.MIN, accum_init=C1,
        reference=lambda in0, in1, s0, s1, imm2: np.where(
            in0 == s0, np.arange(in0.shape[-1], dtype=np.float32), s1),
    ))

    # out = (in0 != s0) + in1   (mismatch + base, fused)
    NEADD = reg("NEADD_ANT", Spec(
        body=ne(Src0, C0) + Src1,
        reference=lambda in0, in1, s0, s1, imm2: (in0 != s0) + in1,
    ))
    return DPSTEP, FIRSTIDX, NEADD


def _build():
    import sys
    if '/opt/trn_rl_repo' not in sys.path:
        sys.path.insert(0, '/opt/trn_rl_repo')
    import concourse.bass as bass
    import concourse.bacc as bacc
    import concourse.mybir as mybir
    import concourse.tile as tile

    DPSTEP, FIRSTIDX, NEADD = _register_custom_ops()

    fp32 = mybir.dt.float32
    fp16 = mybir.dt.float16
    i32 = mybir.dt.int32
    u32 = mybir.dt.uint32
    Alu = mybir.AluOpType
    AX = mybir.AxisListType.X

    nc = bacc.Bacc(None, target_bir_lowering=False, debug=False)
    x = nc.dram_tensor("input", [BC, S, V], fp32, kind="ExternalInput")
    tg = nc.dram_tensor("target", [BC, S], fp32, kind="ExternalInput")
    out = nc.dram_tensor("loss_part", [BC, 1], fp32, kind="ExternalOutput")
    if DEBUG:
        dbg_t = nc.dram_tensor("dbg_t", [128, S], fp32, kind="ExternalOutput")
        dbg_b = nc.dram_tensor("dbg_b", [128, S], fp32, kind="ExternalOutput")
        dbg_i = nc.dram_tensor("dbg_i", [128, NBLK], fp32, kind="ExternalOutput")
        dbg_g = nc.dram_tensor("dbg_g", [BC, 4 * GW], fp32, kind="ExternalOutput")
        dbg_R = nc.dram_tensor("dbg_R", [BC, J1], fp32, kind="ExternalOutput")
    scratch = nc.dram_tensor("rowbase_scratch", [BC, SW], fp32, kind="Internal")

    # refetch view: one row per 128-float sub
    x_subs = x[:, :, :].rearrange("b s (c v) -> (b s c) v", v=SUB)  # [256000,128]

    with tile.TileContext(nc) as tc:
        with tc.tile_pool(name="persist", bufs=1) as cp, \
             tc.tile_pool(name="chunks", bufs=4) as chp, \
             tc.tile_pool(name="work", bufs=2) as wp:

            # ---------------- one-time setup ----------------
            # rowbase[p] = b*64000 + s_local*250  (p = b*32 + s_local)
            T = cp.tile([SW, BC], i32, tag="T")
            nc.gpsimd.iota(T[:, :], pattern=[[S, BC]], base=0,
                           channel_multiplier=1)
            Tf = cp.tile([SW, BC], fp32, tag="Tf")
            nc.vector.tensor_scalar(out=Tf[:, :], in0=T[:, :],
                                    scalar1=float(NSUB), scalar2=None,
                                    op0=Alu.mult)
            nc.sync.dma_start(out=scratch[:, :].rearrange("b s -> s b"),
                              in_=Tf[:, :])
            rowb_f = cp.tile([128, 1], fp32, tag="rowb_f")
            nc.sync.dma_start(
                out=rowb_f[:, :],
                in_=scratch[:, :].rearrange("b (s u) -> (b s) u", u=1))

            # target broadcast [128, 256]: row p=(b, j_local) -> target[b, :]
            t_bcast = cp.tile([128, S], fp32, tag="t_bcast")
            for b in range(BC):
                nc.sync.dma_start(
                    out=t_bcast[SW * b:SW * (b + 1), :],
                    in_=tg[b:b + 1, :].to_broadcast([SW, S]))
            # base[p, i] = 512 - 514*w_i(b)
            wrow = cp.tile([128, S], fp32, tag="wrow")
            nc.vector.tensor_scalar(out=wrow[:, :], in0=t_bcast[:, :],
                                    scalar1=0.0, scalar2=None,
                                    op0=Alu.not_equal)
            base_bc = cp.tile([128, S], fp16, tag="base_bc")
            nc.vector.tensor_scalar(out=base_bc[:, :], in0=wrow[:, :],
                                    scalar1=-514.0, scalar2=BIG,
                                    op0=Alu.mult, op1=Alu.add)

            # G rows [4, 256 * 258] fp16; col0 = col257 = BIG
            G = cp.tile([BC, S * GW], fp16, tag="G")
            G3 = G[:, :].rearrange("p (j i) -> p j i", i=GW)
            nc.vector.memset(G3[:, :, 0:1], BIG)
            nc.vector.memset(G3[:, :, GW - 1:GW], BIG)

            # DP state
            sa = cp.tile([BC, GW], fp16, tag="sa")
            sb = cp.tile([BC, GW], fp16, tag="sb")
            nc.vector.memset(sa[:, :], 0.0)
            nc.vector.memset(sa[:, 0:1], BIG)
            R = cp.tile([BC, J1], fp16, tag="R")
            nc.vector.memset(R[:, 0:1], 0.0)
            cur, nxt = sa, sb

            # per-sigma staged state for the software pipeline
            stage = {}

            def emit_stream(k):
                mall = wp.tile([128, NSUB], fp32, name="mall", tag="mall")
                for c in range(NCH):
                    ch = chp.tile([128, VC], fp32, tag="ch")
                    nc.sync.dma_start(
                        out=ch[:, :],
                        in_=x[:, SW * k:SW * (k + 1), VC * c:VC * (c + 1)])
                    ch3 = ch[:, :].rearrange("p (s v) -> p s v", v=SUB)
                    nc.vector.tensor_reduce(out=mall[:, SPC * c:SPC * (c + 1)],
                                            in_=ch3, axis=AX, op=Alu.max)
                return mall

            def emit_select(k, mall):
                rmax = wp.tile([128, 1], fp32, tag="rmax")
                nc.vector.tensor_reduce(out=rmax[:, :], in_=mall[:, :],
                                        axis=AX, op=Alu.max)
                fsc = wp.tile([128, NSUB], fp32, tag="fsc")
                f = wp.tile([128, 1], fp32, tag="f")
                nc.vector._custom_dve(FIRSTIDX, out=fsc[:, :], in0=mall[:, :],
                                      s0=rmax[:, :1], s1=1000.0,
                                      accum_out=f[:, :])
                fetchf = wp.tile([128, 1], fp32, tag="fetchf")
                nc.vector.tensor_scalar(out=fetchf[:, :], in0=f[:, :],
                                        scalar1=float(k * SW * NSUB),
                                        scalar2=rowb_f[:, :1],
                                        op0=Alu.add, op1=Alu.add)
                fetch = wp.tile([128, 1], i32, tag="fetch")
                nc.vector.tensor_copy(out=fetch[:, :], in_=fetchf[:, :])
                refetch = wp.tile([128, SUB], fp32, tag="refetch")
                nc.gpsimd.indirect_dma_start(
                    out=refetch[:, :], out_offset=None,
                    in_=x_subs[:, :],
                    in_offset=bass.IndirectOffsetOnAxis(ap=fetch[:, :1], axis=0))
                return f, refetch

            def emit_resolve(k, f, refetch):
                m8 = wp.tile([128, 8], fp32, tag="m8")
                nc.vector.max(out=m8[:, :], in_=refetch[:, :])
                i8 = wp.tile([128, 8], u32, tag="i8")
                nc.vector.max_index(out=i8[:, :], in_max=m8[:, :],
                                    in_values=refetch[:, :])
                idxf = wp.tile([128, 1], fp32, tag="idxf")
                nc.vector.tensor_copy(out=idxf[:, :], in_=i8[:, 0:1])
                idxg = wp.tile([128, 1], fp32, tag="idxg")
                nc.vector.tensor_scalar(out=idxg[:, :], in0=f[:, :],
                                        scalar1=float(SUB),
                                        scalar2=idxf[:, :1],
                                        op0=Alu.mult, op1=Alu.add)
                mt = wp.tile([128, S], fp16, tag="mt")
                nc.vector._custom_dve(NEADD, out=mt[:, :], in0=t_bcast[:, :],
                                      in1=base_bc[:, :], s0=idxg[:, :1],
                                      s1=0.0)
                if DEBUG:
                    nc.sync.dma_start(out=dbg_i[:, k:k + 1], in_=idxg[:, :])
                nc.scalar.dma_start(
                    out=G3[:, SW * k:SW * (k + 1), 1:S + 1],
                    in_=mt[:, :])

            def emit_dp(k):
                nonlocal cur, nxt
                for j in range(SW * k + 1, SW * (k + 1) + 1):
                    g = G[:, (j - 1) * GW:(j - 1) * GW + GW]
                    nc.vector._custom_dve(DPSTEP, out=nxt[:, :],
                                          in0=cur[:, :], in1=g,
                                          s0=BIG, s1=0.0,
                                          accum_out=R[:, j:j + 1])
                    cur, nxt = nxt, cur

            # ---------------- pipelined main loop ----------------
            for k in range(NBLK):
                mall = emit_stream(k)
                f, refetch = emit_select(k, mall)
                if k > 0:
                    emit_dp(k - 1)
                emit_resolve(k, f, refetch)
            emit_dp(NBLK - 1)

            if DEBUG:
                dtt = cp.tile([128, S], fp32, tag="dtt")
                nc.vector.tensor_copy(out=dtt[:, :], in_=t_bcast[:, :])
                nc.sync.dma_start(out=dbg_t[:, :], in_=dtt[:, :])
                dbb = cp.tile([128, S], fp32, tag="dbb")
                nc.vector.tensor_copy(out=dbb[:, :], in_=base_bc[:, :])
                nc.sync.dma_start(out=dbg_b[:, :], in_=dbb[:, :])
                dgg = cp.tile([BC, 4 * GW], fp32, tag="dgg")
                for di, j in enumerate((0, 1, 100, 255)):
                    nc.vector.tensor_copy(
                        out=dgg[:, di * GW:(di + 1) * GW],
                        in_=G[:, j * GW:(j + 1) * GW])
                nc.sync.dma_start(out=dbg_g[:, :], in_=dgg[:, :])
                dRR = cp.tile([BC, J1], fp32, tag="dRR")
                nc.vector.tensor_copy(out=dRR[:, :], in_=R[:, :])
                nc.sync.dma_start(out=dbg_R[:, :], in_=dRR[:, :])

            # ---------------- extraction ----------------
            tg4 = cp.tile([BC, S], fp32, tag="tg4")
            nc.sync.dma_start(out=tg4[:, :], in_=tg[:, :])
            w4 = cp.tile([BC, S], fp32, tag="w4")
            nc.vector.tensor_scalar(out=w4[:, :], in0=tg4[:, :],
                                    scalar1=0.0, scalar2=None,
                                    op0=Alu.not_equal)
            lenr = cp.tile([BC, 1], fp32, tag="lenr")
            nc.vector.tensor_reduce(out=lenr[:, :], in_=w4[:, :],
                                    axis=AX, op=Alu.add)
            len2 = cp.tile([BC, 1], fp32, tag="len2")
            nc.vector.tensor_scalar(out=len2[:, :], in0=lenr[:, :],
                                    scalar1=2.0, scalar2=None, op0=Alu.mult)
            iotai = cp.tile([BC, J1], i32, tag="iotai")
            nc.gpsimd.iota(iotai[:, :], pattern=[[1, J1]], base=0,
                           channel_multiplier=0)
            iota_f = cp.tile([BC, J1], fp32, tag="iota_f")
            nc.vector.tensor_copy(out=iota_f[:, :], in_=iotai[:, :])
            eqj = cp.tile([BC, J1], fp32, tag="eqj")
            nc.vector.tensor_scalar(out=eqj[:, :], in0=iota_f[:, :],
                                    scalar1=lenr[:, :1], scalar2=None,
                                    op0=Alu.is_equal)
            Rf = cp.tile([BC, J1], fp32, tag="Rf")
            nc.vector.tensor_copy(out=Rf[:, :], in_=R[:, :])
            prod = cp.tile([BC, J1], fp32, tag="prod")
            nc.vector.tensor_tensor(out=prod[:, :], in0=eqj[:, :],
                                    in1=Rf[:, :], op=Alu.mult)
            red = cp.tile([BC, 1], fp32, tag="red")
            nc.vector.tensor_reduce(out=red[:, :], in_=prod[:, :],
                                    axis=AX, op=Alu.add)
            loss = cp.tile([BC, 1], fp32, tag="loss")
            nc.vector.tensor_scalar(out=loss[:, :], in0=red[:, :],
                                    scalar1=len2[:, :1], scalar2=None,
                                    op0=Alu.add)
            nc.sync.dma_start(out=out[:, :], in_=loss[:, :])

    nc.compile()
    return nc


def kernel(input, target):
    import sys
    if '/opt/trn_rl_repo' not in sys.path:
        sys.path.insert(0, '/opt/trn_rl_repo')
    from concourse.bass_utils import run_bass_kernel_spmd

    if 'nc' not in _cache:
        _cache['nc'] = _build()
    nc = _cache['nc']

    input = np.ascontiguousarray(np.asarray(input, dtype=np.float32))
    target_f = np.asarray(target).astype(np.float32)

    in_maps = []
    for c in range(NCORES):
        in_maps.append({
            "input": input[BC * c:BC * (c + 1)],
            "target": np.ascontiguousarray(target_f[BC * c:BC * (c + 1)]),
        })
    res = run_bass_kernel_spmd(nc, in_maps, core_ids=list(range(NCORES)))
    parts = [res.results[c]["loss_part"][:, 0] for c in range(NCORES)]
    losses = np.concatenate(parts)
    return np.float32(losses.mean())


# revision 10
# speedup vs baseline: 1.1401x; 1.1391x over previous
"""Trainium2 Bass kernel for nn_CERLoss (CER / Levenshtein DP loss).

Strategy (8 NeuronCores, data-parallel over batch; v2 pipelined design):
  - Each core owns 4 batch rows ([4, 256, 32000] fp32 slab).
  - Streaming is s-interleaved: block sigma covers s in [32*sigma, 32*sigma+32)
    for ALL 4 b rows (partition p = b*32 + s_local), so the argmax for pred
    positions arrives in j-order for every batch row simultaneously.
  - Per block: 10 chunk DMAs [128, 3200] fp32; per chunk one 3D sub-reduce
    [128, 25, 128] -> 25 sub-maxes; rowmax + custom FIRSTIDX finds the first
    128-wide sub attaining the row max (exact fp32); indirect-DMA refetch of
    just that sub ([128, 128] fp32, 0.4% extra traffic); max8 + max_index give
    the exact first-occurrence argmax.
  - The DP is scanned over PRED positions j (transposed vs the reference's
    scan over target i; the recurrences are symmetric) so DP steps can run as
    soon as argmax_j is known: full overlap with streaming. Mismatch rows
    G[b, j, i] = (t_i != idx_j) + 512 - 514*w_i are built 32 at a time with a
    custom fused ne-add DVE op and staged into per-b layout via SBUF->SBUF DMA.
  - Each DP step is ONE custom DVE instruction (DPSTEP): a hand-edited uop
    program that computes out[t] = min(state, cur[t], cur[t-1] + G_j[t-1])
    at 1 element/cycle using a CURR_ALU_OUT delay-lane capture, with the
    running min (scan) and the step's final value (accum_out -> R[:, j])
    emitted in the same instruction. S[j][i] = D[j][i] - j - C_i stays
    integral with |.| <= 1536, exact in fp16.
  - loss_row = R[len_b] + 2*len_b; host averages the 32 per-row losses.
  - Software pipelining: DP steps for block sigma-1 are emitted between the
    select and the resolve of block sigma, hiding the refetch DMA latency.
"""

import copy
import numpy as np

B, S, V = 32, 256, 32000
NCORES = 8
BC = B // NCORES            # batch rows per core = 4
SW = 32                     # s-positions per stream block
NBLK = S // SW              # 8 stream blocks
VC = 3200                   # vocab chunk (floats)
NCH = V // VC               # 10 chunks per block
SUB = 128                   # sub-chunk for argmax refetch
NSUB = V // SUB             # 250 subs per (b,s) row
SPC = VC // SUB             # 25 subs per chunk
BIG = 512.0
GW = S + 2                  # 258-wide DP rows (sentinel col 0, pad col 257)
J1 = S + 1                  # 257 memo columns

_cache = {}
DEBUG = False


def _register_custom_ops():
    import concourse.dve_ops as dve_ops
    from concourse.dve_ops import DveOp, DveOpSpec, has_src1
    from concourse.dve_spec import (Spec, Src0, Src1, C0, C1, Idx, scan, minn,
                                    select, eq, ne, AluOp, lower)
    from concourse.dve_uop import AluInp, DelayInp

    def reg(name, spec, edit=None):
        for o in dve_ops.OPS:
            if o.name == name:
                return o
        row = max(dve_ops._SUB_OPCODE_FOR_NAME.values()) + 1
        dve_ops._SUB_OPCODE_FOR_NAME[name] = row
        shas = {}
        for ver in ("v3", "v4"):
            uops = [copy.deepcopy(u) for u in lower(spec, ver=ver)]
            if edit is not None:
                edit(uops)
            tmp = DveOpSpec(name=name, opcode=row, uops=uops,
                            rd1_en=has_src1(spec))
            shas[ver] = tmp.sha(ver)
            dve_ops._COMPILE_CACHE[(name, ver)] = tmp
        op = DveOp(name, spec, subdim=False, uops_sha=shas)
        dve_ops.OPS.append(op)
        dve_ops.CUSTOM_DVE_SPECS[name] = spec
        return op

    # out[t] = state_t = min(state_{t-1}, in0[t], a_{t-1}); a_t = in1[t]+s1;
    # a_{-1} = state_{-1} = s0; accum_out = final state.
    def dpstep_edit(uops):
        seed, steady = uops
        st0 = seed.datapath_config[0]
        st0.op = AluOp.BYPASS
        st0.alu_src0 = AluInp.PREV_DELAY_3
        st0.alu_src1 = AluInp.PREV_DELAY_3
        st0s = steady.datapath_config[0]
        st0s.alu_src0 = AluInp.PREV_DELAY_0
        st0s.alu_src1 = AluInp.PREV_DELAY_1
        st0s.delay[4] = DelayInp.CURR_ALU_OUT
        st0s.delay_enable[4] = 1
        st1s = steady.datapath_config[1]
        st1s.alu_src1 = AluInp.PREV_DELAY_4

    DPSTEP = reg("DPSTEP_ANT", Spec(
        body=scan(AluOp.MIN, minn(Src0, Src1 + C1), init=C0),
        accum=AluOp.MIN, accum_init=C0,
        reference=lambda in0, in1, s0, s1, imm2: in0,
    ), edit=dpstep_edit)

    # first index k where in0[k] == s0 (else s1); accum MIN -> first idx
    FIRSTIDX = reg("FIRSTIDX_ANT", Spec(
        body=select(eq(Src0, C0), Idx, C1),
        accum=AluOp.MIN, accum_init=C1,
        reference=lambda in0, in1, s0, s1, imm2: np.where(
            in0 == s0, np.arange(in0.shape[-1], dtype=np.float32), s1),
    ))

    # out = (in0 != s0) + in1   (mismatch + base, fused)
    NEADD = reg("NEADD_ANT", Spec(
        body=ne(Src0, C0) + Src1,
        reference=lambda in0, in1, s0, s1, imm2: (in0 != s0) + in1,
    ))
    return DPSTEP, FIRSTIDX, NEADD


def _build():
    import sys
    if '/opt/trn_rl_repo' not in sys.path:
        sys.path.insert(0, '/opt/trn_rl_repo')
    import concourse.bass as bass
    import concourse.bacc as bacc
    import concourse.mybir as mybir
    import concourse.tile as tile

    DPSTEP, FIRSTIDX, NEADD = _register_custom_ops()

    fp32 = mybir.dt.float32
    fp16 = mybir.dt.float16
    i32 = mybir.dt.int32
    u32 = mybir.dt.uint32
    Alu = mybir.AluOpType
    AX = mybir.AxisListType.X

    nc = bacc.Bacc(None, target_bir_lowering=False, debug=False)
    x = nc.dram_tensor("input", [BC, S, V], fp32, kind="ExternalInput")
    tg = nc.dram_tensor("target", [BC, S], fp32, kind="ExternalInput")
    out = nc.dram_tensor("loss_part", [BC, 1], fp32, kind="ExternalOutput")
    if DEBUG:
        dbg_t = nc.dram_tensor("dbg_t", [128, S], fp32, kind="ExternalOutput")
        dbg_b = nc.dram_tensor("dbg_b", [128, S], fp32, kind="ExternalOutput")
        dbg_i = nc.dram_tensor("dbg_i", [128, NBLK], fp32, kind="ExternalOutput")
        dbg_g = nc.dram_tensor("dbg_g", [BC, 4 * GW], fp32, kind="ExternalOutput")
        dbg_R = nc.dram_tensor("dbg_R", [BC, J1], fp32, kind="ExternalOutput")
    scratch = nc.dram_tensor("rowbase_scratch", [BC, SW], fp32, kind="Internal")

    # refetch view: one row per 128-float sub
    x_subs = x[:, :, :].rearrange("b s (c v) -> (b s c) v", v=SUB)  # [256000,128]

    with tile.TileContext(nc) as tc:
        with tc.tile_pool(name="persist", bufs=1) as cp, \
             tc.tile_pool(name="chunks", bufs=4) as chp, \
             tc.tile_pool(name="work", bufs=2) as wp:

            # ---------------- one-time setup ----------------
            # rowbase[p] = b*64000 + s_local*250  (p = b*32 + s_local)
            T = cp.tile([SW, BC], i32, tag="T")
            nc.gpsimd.iota(T[:, :], pattern=[[S, BC]], base=0,
                           channel_multiplier=1)
            Tf = cp.tile([SW, BC], fp32, tag="Tf")
            nc.vector.tensor_scalar(out=Tf[:, :], in0=T[:, :],
                                    scalar1=float(NSUB), scalar2=None,
                                    op0=Alu.mult)
            nc.sync.dma_start(out=scratch[:, :].rearrange("b s -> s b"),
                              in_=Tf[:, :])
            rowb_f = cp.tile([128, 1], fp32, tag="rowb_f")
            nc.sync.dma_start(
                out=rowb_f[:, :],
                in_=scratch[:, :].rearrange("b (s u) -> (b s) u", u=1))

            # target broadcast [128, 256]: row p=(b, j_local) -> target[b, :]
            t_bcast = cp.tile([128, S], fp32, tag="t_bcast")
            for b in range(BC):
                nc.sync.dma_start(
                    out=t_bcast[SW * b:SW * (b + 1), :],
                    in_=tg[b:b + 1, :].to_broadcast([SW, S]))
            # base[p, i] = 512 - 514*w_i(b)
            wrow = cp.tile([128, S], fp32, tag="wrow")
            nc.vector.tensor_scalar(out=wrow[:, :], in0=t_bcast[:, :],
                                    scalar1=0.0, scalar2=None,
                                    op0=Alu.not_equal)
            base_bc = cp.tile([128, S], fp16, tag="base_bc")
            nc.vector.tensor_scalar(out=base_bc[:, :], in0=wrow[:, :],
                                    scalar1=-514.0, scalar2=BIG,
                                    op0=Alu.mult, op1=Alu.add)

            # G rows [4, 256 * 258] fp16; col0 = col257 = BIG
            G = cp.tile([BC, S * GW], fp16, tag="G")
            G3 = G[:, :].rearrange("p (j i) -> p j i", i=GW)
            nc.vector.memset(G3[:, :, 0:1], BIG)
            nc.vector.memset(G3[:, :, GW - 1:GW], BIG)

            # DP state
            sa = cp.tile([BC, GW], fp16, tag="sa")
            sb = cp.tile([BC, GW], fp16, tag="sb")
            nc.vector.memset(sa[:, :], 0.0)
            nc.vector.memset(sa[:, 0:1], BIG)
            R = cp.tile([BC, J1], fp16, tag="R")
            nc.vector.memset(R[:, 0:1], 0.0)
            cur, nxt = sa, sb

            # per-sigma staged state for the software pipeline
            stage = {}

            def emit_stream(k):
                mall = wp.tile([128, NSUB], fp32, name="mall", tag="mall")
                for c in range(NCH):
                    ch = chp.tile([128, VC], fp32, tag="ch")
                    nc.gpsimd.dma_start(
                        out=ch[:, :],
                        in_=x[:, SW * k:SW * (k + 1), VC * c:VC * (c + 1)])
                    ch3 = ch[:, :].rearrange("p (s v) -> p s v", v=SUB)
                    nc.vector.tensor_reduce(out=mall[:, SPC * c:SPC * (c + 1)],
                                            in_=ch3, axis=AX, op=Alu.max)
                return mall

            def emit_select(k, mall):
                rmax = wp.tile([128, 1], fp32, tag="rmax")
                nc.vector.tensor_reduce(out=rmax[:, :], in_=mall[:, :],
                                        axis=AX, op=Alu.max)
                fsc = wp.tile([128, NSUB], fp32, tag="fsc")
                f = wp.tile([128, 1], fp32, tag="f")
                nc.vector._custom_dve(FIRSTIDX, out=fsc[:, :], in0=mall[:, :],
                                      s0=rmax[:, :1], s1=1000.0,
                                      accum_out=f[:, :])
                fetchf = wp.tile([128, 1], fp32, tag="fetchf")
                nc.vector.tensor_scalar(out=fetchf[:, :], in0=f[:, :],
                                        scalar1=float(k * SW * NSUB),
                                        scalar2=rowb_f[:, :1],
                                        op0=Alu.add, op1=Alu.add)
                fetch = wp.tile([128, 1], i32, tag="fetch")
                nc.vector.tensor_copy(out=fetch[:, :], in_=fetchf[:, :])
                refetch = wp.tile([128, SUB], fp32, tag="refetch")
                nc.gpsimd.indirect_dma_start(
                    out=refetch[:, :], out_offset=None,
                    in_=x_subs[:, :],
                    in_offset=bass.IndirectOffsetOnAxis(ap=fetch[:, :1], axis=0))
                return f, refetch

            def emit_resolve(k, f, refetch):
                m8 = wp.tile([128, 8], fp32, tag="m8")
                nc.vector.max(out=m8[:, :], in_=refetch[:, :])
                i8 = wp.tile([128, 8], u32, tag="i8")
                nc.vector.max_index(out=i8[:, :], in_max=m8[:, :],
                                    in_values=refetch[:, :])
                idxf = wp.tile([128, 1], fp32, tag="idxf")
                nc.vector.tensor_copy(out=idxf[:, :], in_=i8[:, 0:1])
                idxg = wp.tile([128, 1], fp32, tag="idxg")
                nc.vector.tensor_scalar(out=idxg[:, :], in0=f[:, :],
                                        scalar1=float(SUB),
                                        scalar2=idxf[:, :1],
                                        op0=Alu.mult, op1=Alu.add)
                mt = wp.tile([128, S], fp16, tag="mt")
                nc.vector._custom_dve(NEADD, out=mt[:, :], in0=t_bcast[:, :],
                                      in1=base_bc[:, :], s0=idxg[:, :1],
                                      s1=0.0)
                if DEBUG:
                    nc.sync.dma_start(out=dbg_i[:, k:k + 1], in_=idxg[:, :])
                nc.scalar.dma_start(
                    out=G3[:, SW * k:SW * (k + 1), 1:S + 1],
                    in_=mt[:, :])

            def emit_dp(k):
                nonlocal cur, nxt
                for j in range(SW * k + 1, SW * (k + 1) + 1):
                    g = G[:, (j - 1) * GW:(j - 1) * GW + GW]
                    nc.vector._custom_dve(DPSTEP, out=nxt[:, :],
                                          in0=cur[:, :], in1=g,
                                          s0=BIG, s1=0.0,
                                          accum_out=R[:, j:j + 1])
                    cur, nxt = nxt, cur

            # ---------------- pipelined main loop ----------------
            for k in range(NBLK):
                mall = emit_stream(k)
                f, refetch = emit_select(k, mall)
                if k > 0:
                    emit_dp(k - 1)
                emit_resolve(k, f, refetch)
            emit_dp(NBLK - 1)

            if DEBUG:
                dtt = cp.tile([128, S], fp32, tag="dtt")
                nc.vector.tensor_copy(out=dtt[:, :], in_=t_bcast[:, :])
                nc.sync.dma_start(out=dbg_t[:, :], in_=dtt[:, :])
                dbb = cp.tile([128, S], fp32, tag="dbb")
                nc.vector.tensor_copy(out=dbb[:, :], in_=base_bc[:, :])
                nc.sync.dma_start(out=dbg_b[:, :], in_=dbb[:, :])
                dgg = cp.tile([BC, 4 * GW], fp32, tag="dgg")
                for di, j in enumerate((0, 1, 100, 255)):
                    nc.vector.tensor_copy(
                        out=dgg[:, di * GW:(di + 1) * GW],
                        in_=G[:, j * GW:(j + 1) * GW])
                nc.sync.dma_start(out=dbg_g[:, :], in_=dgg[:, :])
                dRR = cp.tile([BC, J1], fp32, tag="dRR")
                nc.vector.tensor_copy(out=dRR[:, :], in_=R[:, :])
                nc.sync.dma_start(out=dbg_R[:, :], in_=dRR[:, :])

            # ---------------- extraction ----------------
            tg4 = cp.tile([BC, S], fp32, tag="tg4")
            nc.sync.dma_start(out=tg4[:, :], in_=tg[:, :])
            w4 = cp.tile([BC, S], fp32, tag="w4")
            nc.vector.tensor_scalar(out=w4[:, :], in0=tg4[:, :],
                                    scalar1=0.0, scalar2=None,
                                    op0=Alu.not_equal)
            lenr = cp.tile([BC, 1], fp32, tag="lenr")
            nc.vector.tensor_reduce(out=lenr[:, :], in_=w4[:, :],
                                    axis=AX, op=Alu.add)
            len2 = cp.tile([BC, 1], fp32, tag="len2")
            nc.vector.tensor_scalar(out=len2[:, :], in0=lenr[:, :],
                                    scalar1=2.0, scalar2=None, op0=Alu.mult)
            iotai = cp.tile([BC, J1], i32, tag="iotai")
            nc.gpsimd.iota(iotai[:, :], pattern=[[1, J1]], base=0,
                           channel_multiplier=0)
            iota_f = cp.tile([BC, J1], fp32, tag="iota_f")
            nc.vector.tensor_copy(out=iota_f[:, :], in_=iotai[:, :])
            eqj = cp.tile([BC, J1], fp32, tag="eqj")
            nc.vector.tensor_scalar(out=eqj[:, :], in0=iota_f[:, :],
                                    scalar1=lenr[:, :1], scalar2=None,
                                    op0=Alu.is_equal)
            Rf = cp.tile([BC, J1], fp32, tag="Rf")
            nc.vector.tensor_copy(out=Rf[:, :], in_=R[:, :])
            prod = cp.tile([BC, J1], fp32, tag="prod")
            nc.vector.tensor_tensor(out=prod[:, :], in0=eqj[:, :],
                                    in1=Rf[:, :], op=Alu.mult)
            red = cp.tile([BC, 1], fp32, tag="red")
            nc.vector.tensor_reduce(out=red[:, :], in_=prod[:, :],
                                    axis=AX, op=Alu.add)
            loss = cp.tile([BC, 1], fp32, tag="loss")
            nc.vector.tensor_scalar(out=loss[:, :], in0=red[:, :],
                                    scalar1=len2[:, :1], scalar2=None,
                                    op0=Alu.add)
            nc.sync.dma_start(out=out[:, :], in_=loss[:, :])

    nc.compile()
    return nc


def kernel(input, target):
    import sys
    if '/opt/trn_rl_repo' not in sys.path:
        sys.path.insert(0, '/opt/trn_rl_repo')
    from concourse.bass_utils import run_bass_kernel_spmd

    if 'nc' not in _cache:
        _cache['nc'] = _build()
    nc = _cache['nc']

    input = np.ascontiguousarray(np.asarray(input, dtype=np.float32))
    target_f = np.asarray(target).astype(np.float32)

    in_maps = []
    for c in range(NCORES):
        in_maps.append({
            "input": input[BC * c:BC * (c + 1)],
            "target": np.ascontiguousarray(target_f[BC * c:BC * (c + 1)]),
        })
    res = run_bass_kernel_spmd(nc, in_maps, core_ids=list(range(NCORES)))
    parts = [res.results[c]["loss_part"][:, 0] for c in range(NCORES)]
    losses = np.concatenate(parts)
    return np.float32(losses.mean())


# revision 11
# speedup vs baseline: 1.9513x; 1.7116x over previous
"""Trainium2 Bass kernel for nn_CERLoss (CER / Levenshtein DP loss).

Strategy (8 NeuronCores, data-parallel over batch; v2 pipelined design):
  - Each core owns 4 batch rows ([4, 256, 32000] fp32 slab).
  - Streaming is s-interleaved: block sigma covers s in [32*sigma, 32*sigma+32)
    for ALL 4 b rows (partition p = b*32 + s_local), so the argmax for pred
    positions arrives in j-order for every batch row simultaneously.
  - Per block: 10 chunk DMAs [128, 3200] fp32; per chunk one 3D sub-reduce
    [128, 25, 128] -> 25 sub-maxes; rowmax + custom FIRSTIDX finds the first
    128-wide sub attaining the row max (exact fp32); indirect-DMA refetch of
    just that sub ([128, 128] fp32, 0.4% extra traffic); max8 + max_index give
    the exact first-occurrence argmax.
  - The DP is scanned over PRED positions j (transposed vs the reference's
    scan over target i; the recurrences are symmetric) so DP steps can run as
    soon as argmax_j is known: full overlap with streaming. Mismatch rows
    G[b, j, i] = (t_i != idx_j) + 512 - 514*w_i are built 32 at a time with a
    custom fused ne-add DVE op and staged into per-b layout via SBUF->SBUF DMA.
  - Each DP step is ONE custom DVE instruction (DPSTEP): a hand-edited uop
    program that computes out[t] = min(state, cur[t], cur[t-1] + G_j[t-1])
    at 1 element/cycle using a CURR_ALU_OUT delay-lane capture, with the
    running min (scan) and the step's final value (accum_out -> R[:, j])
    emitted in the same instruction. S[j][i] = D[j][i] - j - C_i stays
    integral with |.| <= 1536, exact in fp16.
  - loss_row = R[len_b] + 2*len_b; host averages the 32 per-row losses.
  - Software pipelining: DP steps for block sigma-1 are emitted between the
    select and the resolve of block sigma, hiding the refetch DMA latency.
"""

import copy
import numpy as np

B, S, V = 32, 256, 32000
NCORES = 8
BC = B // NCORES            # batch rows per core = 4
SW = 32                     # s-positions per stream block
NBLK = S // SW              # 8 stream blocks
VC = 3200                   # vocab chunk (floats)
NCH = V // VC               # 10 chunks per block
SUB = 128                   # sub-chunk for argmax refetch
NSUB = V // SUB             # 250 subs per (b,s) row
SPC = VC // SUB             # 25 subs per chunk
BIG = 512.0
GW = S + 2                  # 258-wide DP rows (sentinel col 0, pad col 257)
J1 = S + 1                  # 257 memo columns

_cache = {}
DEBUG = False


def _register_custom_ops():
    import concourse.dve_ops as dve_ops
    from concourse.dve_ops import DveOp, DveOpSpec, has_src1
    from concourse.dve_spec import (Spec, Src0, Src1, C0, C1, Idx, scan, minn,
                                    select, eq, ne, AluOp, lower)
    from concourse.dve_uop import AluInp, DelayInp

    def reg(name, spec, edit=None):
        for o in dve_ops.OPS:
            if o.name == name:
                return o
        row = max(dve_ops._SUB_OPCODE_FOR_NAME.values()) + 1
        dve_ops._SUB_OPCODE_FOR_NAME[name] = row
        shas = {}
        for ver in ("v3", "v4"):
            uops = [copy.deepcopy(u) for u in lower(spec, ver=ver)]
            if edit is not None:
                edit(uops)
            tmp = DveOpSpec(name=name, opcode=row, uops=uops,
                            rd1_en=has_src1(spec))
            shas[ver] = tmp.sha(ver)
            dve_ops._COMPILE_CACHE[(name, ver)] = tmp
        op = DveOp(name, spec, subdim=False, uops_sha=shas)
        dve_ops.OPS.append(op)
        dve_ops.CUSTOM_DVE_SPECS[name] = spec
        return op

    # out[t] = state_t = min(state_{t-1}, in0[t], a_{t-1}); a_t = in1[t]+s1;
    # a_{-1} = state_{-1} = s0; accum_out = final state.
    def dpstep_edit(uops):
        seed, steady = uops
        st0 = seed.datapath_config[0]
        st0.op = AluOp.BYPASS
        st0.alu_src0 = AluInp.PREV_DELAY_3
        st0.alu_src1 = AluInp.PREV_DELAY_3
        st0s = steady.datapath_config[0]
        st0s.alu_src0 = AluInp.PREV_DELAY_0
        st0s.alu_src1 = AluInp.PREV_DELAY_1
        st0s.delay[4] = DelayInp.CURR_ALU_OUT
        st0s.delay_enable[4] = 1
        st1s = steady.datapath_config[1]
        st1s.alu_src1 = AluInp.PREV_DELAY_4

    DPSTEP = reg("DPSTEP_ANT", Spec(
        body=scan(AluOp.MIN, minn(Src0, Src1 + C1), init=C0),
        accum=AluOp.MIN, accum_init=C0,
        reference=lambda in0, in1, s0, s1, imm2: in0,
    ), edit=dpstep_edit)

    # first index k where in0[k] == s0 (else s1); accum MIN -> first idx
    FIRSTIDX = reg("FIRSTIDX_ANT", Spec(
        body=select(eq(Src0, C0), Idx, C1),
        accum=AluOp.MIN, accum_init=C1,
        reference=lambda in0, in1, s0, s1, imm2: np.where(
            in0 == s0, np.arange(in0.shape[-1], dtype=np.float32), s1),
    ))

    # out = (in0 != s0) + in1   (mismatch + base, fused)
    NEADD = reg("NEADD_ANT", Spec(
        body=ne(Src0, C0) + Src1,
        reference=lambda in0, in1, s0, s1, imm2: (in0 != s0) + in1,
    ))
    return DPSTEP, FIRSTIDX, NEADD


def _build():
    import sys
    if '/opt/trn_rl_repo' not in sys.path:
        sys.path.insert(0, '/opt/trn_rl_repo')
    import concourse.bass as bass
    import concourse.bacc as bacc
    import concourse.mybir as mybir
    import concourse.tile as tile

    DPSTEP, FIRSTIDX, NEADD = _register_custom_ops()

    fp32 = mybir.dt.float32
    fp16 = mybir.dt.float16
    i32 = mybir.dt.int32
    u32 = mybir.dt.uint32
    Alu = mybir.AluOpType
    AX = mybir.AxisListType.X

    nc = bacc.Bacc(None, target_bir_lowering=False, debug=False)
    x = nc.dram_tensor("input", [BC, S, V], fp32, kind="ExternalInput")
    tg = nc.dram_tensor("target", [BC, S], fp32, kind="ExternalInput")
    out = nc.dram_tensor("loss_part", [BC, 1], fp32, kind="ExternalOutput")
    if DEBUG:
        dbg_t = nc.dram_tensor("dbg_t", [128, S], fp32, kind="ExternalOutput")
        dbg_b = nc.dram_tensor("dbg_b", [128, S], fp32, kind="ExternalOutput")
        dbg_i = nc.dram_tensor("dbg_i", [128, NBLK], fp32, kind="ExternalOutput")
        dbg_g = nc.dram_tensor("dbg_g", [BC, 4 * GW], fp32, kind="ExternalOutput")
        dbg_R = nc.dram_tensor("dbg_R", [BC, J1], fp32, kind="ExternalOutput")
    scratch = nc.dram_tensor("rowbase_scratch", [BC, SW], fp32, kind="Internal")

    # refetch view: one row per 128-float sub
    x_subs = x[:, :, :].rearrange("b s (c v) -> (b s c) v", v=SUB)  # [256000,128]

    with tile.TileContext(nc) as tc:
        with tc.tile_pool(name="persist", bufs=1) as cp, \
             tc.tile_pool(name="chunks", bufs=4) as chp, \
             tc.tile_pool(name="work", bufs=2) as wp:

            # ---------------- one-time setup ----------------
            # rowbase[p] = b*64000 + s_local*250  (p = b*32 + s_local)
            T = cp.tile([SW, BC], i32, tag="T")
            nc.gpsimd.iota(T[:, :], pattern=[[S, BC]], base=0,
                           channel_multiplier=1)
            Tf = cp.tile([SW, BC], fp32, tag="Tf")
            nc.vector.tensor_scalar(out=Tf[:, :], in0=T[:, :],
                                    scalar1=float(NSUB), scalar2=None,
                                    op0=Alu.mult)
            nc.sync.dma_start(out=scratch[:, :].rearrange("b s -> s b"),
                              in_=Tf[:, :])
            rowb_f = cp.tile([128, 1], fp32, tag="rowb_f")
            nc.sync.dma_start(
                out=rowb_f[:, :],
                in_=scratch[:, :].rearrange("b (s u) -> (b s) u", u=1))

            # target broadcast [128, 256]: row p=(b, j_local) -> target[b, :]
            t_bcast = cp.tile([128, S], fp32, tag="t_bcast")
            for b in range(BC):
                nc.sync.dma_start(
                    out=t_bcast[SW * b:SW * (b + 1), :],
                    in_=tg[b:b + 1, :].to_broadcast([SW, S]))
            # base[p, i] = 512 - 514*w_i(b)
            wrow = cp.tile([128, S], fp32, tag="wrow")
            nc.vector.tensor_scalar(out=wrow[:, :], in0=t_bcast[:, :],
                                    scalar1=0.0, scalar2=None,
                                    op0=Alu.not_equal)
            base_bc = cp.tile([128, S], fp16, tag="base_bc")
            nc.vector.tensor_scalar(out=base_bc[:, :], in0=wrow[:, :],
                                    scalar1=-514.0, scalar2=BIG,
                                    op0=Alu.mult, op1=Alu.add)

            # G rows [4, 256 * 258] fp16; col0 = col257 = BIG
            G = cp.tile([BC, S * GW], fp16, tag="G")
            G3 = G[:, :].rearrange("p (j i) -> p j i", i=GW)
            nc.vector.memset(G3[:, :, 0:1], BIG)
            nc.vector.memset(G3[:, :, GW - 1:GW], BIG)

            # DP state
            sa = cp.tile([BC, GW], fp16, tag="sa")
            sb = cp.tile([BC, GW], fp16, tag="sb")
            nc.vector.memset(sa[:, :], 0.0)
            nc.vector.memset(sa[:, 0:1], BIG)
            R = cp.tile([BC, J1], fp16, tag="R")
            nc.vector.memset(R[:, 0:1], 0.0)
            cur, nxt = sa, sb

            # per-sigma staged state for the software pipeline
            stage = {}

            def emit_stream(k):
                mall = wp.tile([128, NSUB], fp32, name="mall", tag="mall")
                for c in range(NCH):
                    ch = chp.tile([128, VC], fp32, tag="ch")
                    for b in range(BC):
                        eng = nc.sync if b < 2 else nc.scalar
                        eng.dma_start(
                            out=ch[SW * b:SW * (b + 1), :],
                            in_=x[b:b + 1, SW * k:SW * (k + 1),
                                  VC * c:VC * (c + 1)])
                    ch3 = ch[:, :].rearrange("p (s v) -> p s v", v=SUB)
                    nc.vector.tensor_reduce(out=mall[:, SPC * c:SPC * (c + 1)],
                                            in_=ch3, axis=AX, op=Alu.max)
                return mall

            def emit_select(k, mall):
                rmax = wp.tile([128, 1], fp32, tag="rmax")
                nc.vector.tensor_reduce(out=rmax[:, :], in_=mall[:, :],
                                        axis=AX, op=Alu.max)
                fsc = wp.tile([128, NSUB], fp32, tag="fsc")
                f = wp.tile([128, 1], fp32, tag="f")
                nc.vector._custom_dve(FIRSTIDX, out=fsc[:, :], in0=mall[:, :],
                                      s0=rmax[:, :1], s1=1000.0,
                                      accum_out=f[:, :])
                fetchf = wp.tile([128, 1], fp32, tag="fetchf")
                nc.vector.tensor_scalar(out=fetchf[:, :], in0=f[:, :],
                                        scalar1=float(k * SW * NSUB),
                                        scalar2=rowb_f[:, :1],
                                        op0=Alu.add, op1=Alu.add)
                fetch = wp.tile([128, 1], i32, tag="fetch")
                nc.vector.tensor_copy(out=fetch[:, :], in_=fetchf[:, :])
                refetch = wp.tile([128, SUB], fp32, tag="refetch")
                nc.gpsimd.indirect_dma_start(
                    out=refetch[:, :], out_offset=None,
                    in_=x_subs[:, :],
                    in_offset=bass.IndirectOffsetOnAxis(ap=fetch[:, :1], axis=0))
                return f, refetch

            def emit_resolve(k, f, refetch):
                m8 = wp.tile([128, 8], fp32, tag="m8")
                nc.vector.max(out=m8[:, :], in_=refetch[:, :])
                i8 = wp.tile([128, 8], u32, tag="i8")
                nc.vector.max_index(out=i8[:, :], in_max=m8[:, :],
                                    in_values=refetch[:, :])
                idxf = wp.tile([128, 1], fp32, tag="idxf")
                nc.vector.tensor_copy(out=idxf[:, :], in_=i8[:, 0:1])
                idxg = wp.tile([128, 1], fp32, tag="idxg")
                nc.vector.tensor_scalar(out=idxg[:, :], in0=f[:, :],
                                        scalar1=float(SUB),
                                        scalar2=idxf[:, :1],
                                        op0=Alu.mult, op1=Alu.add)
                mt = wp.tile([128, S], fp16, tag="mt")
                nc.vector._custom_dve(NEADD, out=mt[:, :], in0=t_bcast[:, :],
                                      in1=base_bc[:, :], s0=idxg[:, :1],
                                      s1=0.0)
                if DEBUG:
                    nc.sync.dma_start(out=dbg_i[:, k:k + 1], in_=idxg[:, :])
                nc.scalar.dma_start(
                    out=G3[:, SW * k:SW * (k + 1), 1:S + 1],
                    in_=mt[:, :])

            def emit_dp(k):
                nonlocal cur, nxt
                for j in range(SW * k + 1, SW * (k + 1) + 1):
                    g = G[:, (j - 1) * GW:(j - 1) * GW + GW]
                    nc.vector._custom_dve(DPSTEP, out=nxt[:, :],
                                          in0=cur[:, :], in1=g,
                                          s0=BIG, s1=0.0,
                                          accum_out=R[:, j:j + 1])
                    cur, nxt = nxt, cur

            # ---------------- pipelined main loop ----------------
            for k in range(NBLK):
                mall = emit_stream(k)
                f, refetch = emit_select(k, mall)
                if k > 0:
                    emit_dp(k - 1)
                emit_resolve(k, f, refetch)
            emit_dp(NBLK - 1)

            if DEBUG:
                dtt = cp.tile([128, S], fp32, tag="dtt")
                nc.vector.tensor_copy(out=dtt[:, :], in_=t_bcast[:, :])
                nc.sync.dma_start(out=dbg_t[:, :], in_=dtt[:, :])
                dbb = cp.tile([128, S], fp32, tag="dbb")
                nc.vector.tensor_copy(out=dbb[:, :], in_=base_bc[:, :])
                nc.sync.dma_start(out=dbg_b[:, :], in_=dbb[:, :])
                dgg = cp.tile([BC, 4 * GW], fp32, tag="dgg")
                for di, j in enumerate((0, 1, 100, 255)):
                    nc.vector.tensor_copy(
                        out=dgg[:, di * GW:(di + 1) * GW],
                        in_=G[:, j * GW:(j + 1) * GW])
                nc.sync.dma_start(out=dbg_g[:, :], in_=dgg[:, :])
                dRR = cp.tile([BC, J1], fp32, tag="dRR")
                nc.vector.tensor_copy(out=dRR[:, :], in_=R[:, :])
                nc.sync.dma_start(out=dbg_R[:, :], in_=dRR[:, :])

            # ---------------- extraction ----------------
            tg4 = cp.tile([BC, S], fp32, tag="tg4")
            nc.sync.dma_start(out=tg4[:, :], in_=tg[:, :])
            w4 = cp.tile([BC, S], fp32, tag="w4")
            nc.vector.tensor_scalar(out=w4[:, :], in0=tg4[:, :],
                                    scalar1=0.0, scalar2=None,
                                    op0=Alu.not_equal)
            lenr = cp.tile([BC, 1], fp32, tag="lenr")
            nc.vector.tensor_reduce(out=lenr[:, :], in_=w4[:, :],
                                    axis=AX, op=Alu.add)
            len2 = cp.tile([BC, 1], fp32, tag="len2")
            nc.vector.tensor_scalar(out=len2[:, :], in0=lenr[:, :],
                                    scalar1=2.0, scalar2=None, op0=Alu.mult)
            iotai = cp.tile([BC, J1], i32, tag="iotai")
            nc.gpsimd.iota(iotai[:, :], pattern=[[1, J1]], base=0,
                           channel_multiplier=0)
            iota_f = cp.tile([BC, J1], fp32, tag="iota_f")
            nc.vector.tensor_copy(out=iota_f[:, :], in_=iotai[:, :])
            eqj = cp.tile([BC, J1], fp32, tag="eqj")
            nc.vector.tensor_scalar(out=eqj[:, :], in0=iota_f[:, :],
                                    scalar1=lenr[:, :1], scalar2=None,
                                    op0=Alu.is_equal)
            Rf = cp.tile([BC, J1], fp32, tag="Rf")
            nc.vector.tensor_copy(out=Rf[:, :], in_=R[:, :])
            prod = cp.tile([BC, J1], fp32, tag="prod")
            nc.vector.tensor_tensor(out=prod[:, :], in0=eqj[:, :],
                                    in1=Rf[:, :], op=Alu.mult)
            red = cp.tile([BC, 1], fp32, tag="red")
            nc.vector.tensor_reduce(out=red[:, :], in_=prod[:, :],
                                    axis=AX, op=Alu.add)
            loss = cp.tile([BC, 1], fp32, tag="loss")
            nc.vector.tensor_scalar(out=loss[:, :], in0=red[:, :],
                                    scalar1=len2[:, :1], scalar2=None,
                                    op0=Alu.add)
            nc.sync.dma_start(out=out[:, :], in_=loss[:, :])

    nc.compile()
    return nc


def kernel(input, target):
    import sys
    if '/opt/trn_rl_repo' not in sys.path:
        sys.path.insert(0, '/opt/trn_rl_repo')
    from concourse.bass_utils import run_bass_kernel_spmd

    if 'nc' not in _cache:
        _cache['nc'] = _build()
    nc = _cache['nc']

    input = np.ascontiguousarray(np.asarray(input, dtype=np.float32))
    target_f = np.asarray(target).astype(np.float32)

    in_maps = []
    for c in range(NCORES):
        in_maps.append({
            "input": input[BC * c:BC * (c + 1)],
            "target": np.ascontiguousarray(target_f[BC * c:BC * (c + 1)]),
        })
    res = run_bass_kernel_spmd(nc, in_maps, core_ids=list(range(NCORES)))
    parts = [res.results[c]["loss_part"][:, 0] for c in range(NCORES)]
    losses = np.concatenate(parts)
    return np.float32(losses.mean())
